# revision 17
# baseline (speedup 1.0000x reference)
"""GaussianImage rasterization kernel for Trainium2 (8 NeuronCores).

Math: out(h,w,c) = rgb[-1,c]*alpha[-1] * S(h,w),
      S = sum_n exp(-0.5 (p-m_n)^T InvCov_n (p-m_n))

The exponent is expanded into a 6-feature dot product per (gaussian, pixel):
  expo = g1*(4x'^2) + g2*(4x'y') + g3*(4y'^2) + g4*(2x') + g5*(2y') + g6
with x' = x-0.5, y' = y-0.5 (centering improves conditioning).

Each core rasterizes 64 image rows (32768 pixels) against all 128 gaussians:
  - 256 pixel-blocks of 128 pixels; block j holds pixels {q*256+j, q=0..127}
  - matmul: lhsT = fp16 feature rows (K=32, M=128 px), rhs = fp16 coeff rows
    (K=32, N=128 gaussians) -> PSUM (128 px, 128 gaussians) fp32
  - fp16 hi/mid/lo 3-way splitting of both operands gives ~fp32-accurate
    products (paired rows; K time-free on the PE, only N matters)
  - ScalarE: exp over (128, 2048) PSUM -> fp16 SBUF
  - VectorE: scalar_tensor_tensor(a+b) with accum_out -> per-pixel sums
  - 3x tensor_scalar channel scaling, one contiguous DMA out per core
"""

import numpy as np

N_GAUSS = 128
H = 512
W = 512
N_CORES = 8
ROWS_PER_CORE = H // N_CORES          # 64
PX_PER_CORE = ROWS_PER_CORE * W       # 32768
N_BLOCKS = PX_PER_CORE // 128         # 256 blocks of 128 px
N_ROUNDS = N_BLOCKS // 16             # 16 rounds x 16 blocks
N_SLOTS = N_BLOCKS // 4               # 64 slots per row-group

# ---------------------------------------------------------------------------
# Host-side math (fp64): coefficients, features, fp16 splitting
# ---------------------------------------------------------------------------

def _f16_split3(v):
    """Split fp64 array into fp16 hi, mid, lo with v ~ hi+mid+lo."""
    hi = v.astype(np.float16)
    r1 = v - hi.astype(np.float64)
    mid = r1.astype(np.float16)
    r2 = r1 - mid.astype(np.float64)
    lo = r2.astype(np.float16)
    return hi, mid, lo


def _coeffs(mean, scale, theta):
    """Per-gaussian coefficients g1..g6 (fp64), feature-scaled."""
    m = mean.astype(np.float64)
    s = scale.astype(np.float64)
    th = (1.0 + np.sin(theta.astype(np.float64)[:, 0])) * np.pi
    c, sn = np.cos(th), np.sin(th)
    is1 = 1.0 / s[:, 0] ** 2
    is2 = 1.0 / s[:, 1] ** 2
    A = c * c * is1 + sn * sn * is2
    B = c * sn * (is1 - is2)
    C = sn * sn * is1 + c * c * is2
    mx = m[:, 0] - 0.5
    my = m[:, 1] - 0.5
    # features are [4x'^2, 4x'y', 4y'^2, 2x', 2y', 1]
    g = np.stack([
        -A / 8.0,
        -B / 4.0,
        -C / 8.0,
        (A * mx + B * my) / 2.0,
        (B * mx + C * my) / 2.0,
        -0.5 * (A * mx * mx + 2.0 * B * mx * my + C * my * my),
    ], axis=0)  # (6, N)
    return g


def _features(pixels_flat):
    """Feature rows (6, P) fp64 from pixel coords (P, 2)."""
    p = pixels_flat.astype(np.float64)
    x = p[:, 0] - 0.5
    y = p[:, 1] - 0.5
    return np.stack([4*x*x, 4*x*y, 4*y*y, 2*x, 2*y, np.ones_like(x)], axis=0)


# Paired rows: (feature_index, f_piece, g_piece); pieces: 0=hi 1=mid 2=lo.
# 5 "big" features x 6 pairings + const x 2 = 32 rows. hh rows first
# (largest magnitudes accumulate/cancel early in the fp32 PSUM chain).
def _row_plan():
    plan = []
    big = [2, 1, 4, 0, 3]  # y^2, xy, y, x^2, x  (largest |g*f| first)
    for f in big:
        plan.append((f, 0, 0))   # hh
    plan.append((5, 0, 0))       # const * g_hi
    plan.append((5, 0, 1))       # const * g_mid
    for f in big:
        plan.append((f, 0, 1))   # hm
        plan.append((f, 1, 0))   # mh
    for f in big:
        plan.append((f, 1, 1))   # mm
        plan.append((f, 0, 2))   # hl
        plan.append((f, 2, 0))   # lh
    assert len(plan) == 32
    return plan


def _host_prep(mean, rgb, alpha, scale, theta, pixels):
    """Build per-core device operands."""
    plan = _row_plan()
    g = _coeffs(mean, scale, theta)              # (6, 128) fp64
    g_pieces = [_f16_split3(g[f]) for f in range(6)]   # list of (hi,mid,lo)

    # coef rows (32, 128) fp16
    coef = np.stack([g_pieces[f][gp] for (f, _fp, gp) in plan],
                    axis=0).astype(np.float16)

    rgba = (rgb[-1].astype(np.float64) * alpha[-1, 0].astype(np.float64))
    rgba_b = np.zeros((128, 4), dtype=np.float32)
    rgba_b[:, :3] = rgba.astype(np.float32)[None, :]

    # Pixel-block layout: within a core's 32768 pixels (p = q*256 + j),
    # block j holds pixels {q*256+j : q}.  F_sb[k, j*128+q] = F32[k, q*256+j].
    pix = np.asarray(pixels).reshape(H * W, 2)
    feats = []
    for core in range(N_CORES):
        pf = pix[core * PX_PER_CORE:(core + 1) * PX_PER_CORE]
        F = _features(pf)                        # (6, 32768) fp64
        f_pieces = [_f16_split3(F[f]) for f in range(6)]
        F32 = np.stack([f_pieces[f][fp] for (f, fp, _gp) in plan], axis=0)
        Fb = F32.reshape(32, 128, 256)           # [k, q, j]
        Fb = Fb.transpose(0, 2, 1)               # [k, j, q]
        Fsb = Fb.reshape(32, 256 * 128)          # partition k, col j*128+q
        feats.append(np.ascontiguousarray(Fsb.astype(np.float16)))
    return feats, coef, rgba_b


# ---------------------------------------------------------------------------
# Device kernel
# ---------------------------------------------------------------------------

_CACHE = {}


def _build_bass():
    import concourse.bacc as bacc
    import concourse.mybir as mybir
    from concourse.tile import TileContext

    fp16 = mybir.dt.float16
    f32 = mybir.dt.float32

    nc = bacc.Bacc("TRN2", target_bir_lowering=False)
    feat_d = [
        nc.dram_tensor(f"feat{t}", [32, 2048], fp16, kind="ExternalInput")
        for t in range(16)
    ]
    coef_d = nc.dram_tensor("coef", [32, 128], fp16, kind="ExternalInput")
    rgba_d = nc.dram_tensor("rgba", [128, 4], f32, kind="ExternalInput")
    out_d = nc.dram_tensor("out", [128, 768], f32, kind="ExternalOutput")

    with TileContext(nc) as tc:
        with (
            tc.tile_pool(name="const", bufs=1) as cpool,
            tc.tile_pool(name="feat", bufs=1) as fpool,
            tc.tile_pool(name="psum", bufs=2, space="PSUM") as ppool,
            tc.tile_pool(name="splat", bufs=2) as spool,
            tc.tile_pool(name="scratch", bufs=2) as scpool,
            tc.tile_pool(name="acc", bufs=1) as apool,
        ):
            # warm the exp table while DMAs stream
            dummy = cpool.tile([128, 1], fp16, tag="dummy")
            nc.gpsimd.memset(dummy[:], 0)
            nc.scalar.activation(dummy[:], dummy[:],
                                 mybir.ActivationFunctionType.Exp)

            g_sb = cpool.tile([32, 128], fp16, tag="gsb")
            nc.sync.dma_start(g_sb[:], coef_d[:])
            rgba_sb = cpool.tile([128, 4], f32, tag="rgba")
            nc.sync.dma_start(rgba_sb[:], rgba_d[:])

            ftiles = []
            for t in range(16):
                ft = fpool.tile([32, 2048], fp16, tag=f"ft{t}")
                nc.sync.dma_start(ft[:], feat_d[t][:])
                ftiles.append(ft)

            S_big = apool.tile([128, 256], f32, tag="sbig")
            out_big = apool.tile([128, 768], f32, tag="outbig")

            for r in range(N_ROUNDS):
                ps = ppool.tile([128, 2048], f32, tag="ps")
                for i in range(16):
                    nc.tensor.matmul(
                        ps[:, i * 128:(i + 1) * 128],
                        ftiles[r][:, i * 128:(i + 1) * 128],
                        g_sb[:],
                    )
                sp = spool.tile([128, 2048], fp16, tag="sp")
                nc.scalar.activation(sp[:], ps[:],
                                     mybir.ActivationFunctionType.Exp)
                sp3 = sp[:].rearrange("p (i g) -> p i g", g=128)
                sc = scpool.tile([128, 1024], fp16, tag="sc")
                sc3 = sc[:].rearrange("p (i g) -> p i g", g=64)
                nc.gpsimd.tensor_tensor(
                    sc3, sp3[:, :, 0:64], sp3[:, :, 64:128],
                    op=mybir.AluOpType.add,
                )
                nc.vector.tensor_reduce(
                    S_big[:, 16 * r:16 * (r + 1)], sc3,
                    axis=mybir.AxisListType.X, op=mybir.AluOpType.add,
                )

            ob3 = out_big[:].rearrange("p (j c) -> p j c", c=3)
            for c in range(3):
                nc.vector.tensor_scalar_mul(
                    ob3[:, :, c], S_big[:], rgba_sb[:, c:c + 1]
                )
            nc.sync.dma_start(out_d[:], out_big[:])

    nc.finalize()
    return nc


def _run(inputs, trace=False):
    from concourse.bass_utils import run_bass_kernel_spmd

    feats, coef, rgba_b = _host_prep(**inputs)
    if "nc" not in _CACHE:
        _CACHE["nc"] = _build_bass()
    nc = _CACHE["nc"]

    in_maps = []
    for core in range(N_CORES):
        m = {f"feat{t}": np.ascontiguousarray(
                feats[core][:, t * 2048:(t + 1) * 2048]) for t in range(16)}
        m["coef"] = coef
        m["rgba"] = rgba_b
        in_maps.append(m)

    res = run_bass_kernel_spmd(
        nc, in_maps, core_ids=list(range(N_CORES)), trace=trace,
    )
    shards = []
    for core in range(N_CORES):
        o = res.results[core]["out"]           # (128, 768) f32
        o = o.reshape(128, 256, 3)             # [q, j, c]
        o = o.reshape(64, 2, 256, 3)           # [row, half, j, c]
        shards.append(o.reshape(64, 512, 3))
    full = np.concatenate(shards, axis=0).astype(np.float32)
    return full, res


def kernel(mean, rgb, alpha, scale, theta, pixels):
    out, _ = _run(dict(mean=mean, rgb=rgb, alpha=alpha, scale=scale,
                       theta=theta, pixels=pixels))
    return out


# revision 23
# speedup vs baseline: 1.0029x; 1.0029x over previous
"""GaussianImage rasterization kernel for Trainium2 (8 NeuronCores).

Math: out(h,w,c) = rgb[-1,c]*alpha[-1] * S(h,w),
      S = sum_n exp(-0.5 (p-m_n)^T InvCov_n (p-m_n))

The exponent is expanded into a 6-feature dot product per (gaussian, pixel):
  expo = g1*(4x'^2) + g2*(4x'y') + g3*(4y'^2) + g4*(2x') + g5*(2y') + g6
with x' = x-0.5, y' = y-0.5 (centering improves conditioning).

Each core rasterizes 64 image rows (32768 pixels) against all 128 gaussians:
  - 256 pixel-blocks of 128 pixels; block j holds pixels {q*256+j, q=0..127}
  - matmul: lhsT = fp16 feature rows (K=32, M=128 px), rhs = fp16 coeff rows
    (K=32, N=128 gaussians) -> PSUM (128 px, 128 gaussians) fp32
  - fp16 hi/mid/lo 3-way splitting of both operands gives ~fp32-accurate
    products (paired rows; K time-free on the PE, only N matters)
  - ScalarE: exp over (128, 2048) PSUM -> fp16 SBUF
  - VectorE: scalar_tensor_tensor(a+b) with accum_out -> per-pixel sums
  - 3x tensor_scalar channel scaling, one contiguous DMA out per core
"""

import numpy as np

N_GAUSS = 128
H = 512
W = 512
N_CORES = 8
ROWS_PER_CORE = H // N_CORES          # 64
PX_PER_CORE = ROWS_PER_CORE * W       # 32768
N_BLOCKS = PX_PER_CORE // 128         # 256 blocks of 128 px
N_ROUNDS = N_BLOCKS // 16             # 16 rounds x 16 blocks
N_SLOTS = N_BLOCKS // 4               # 64 slots per row-group

# ---------------------------------------------------------------------------
# Host-side math (fp64): coefficients, features, fp16 splitting
# ---------------------------------------------------------------------------

def _f16_split3(v):
    """Split fp64 array into fp16 hi, mid, lo with v ~ hi+mid+lo."""
    hi = v.astype(np.float16)
    r1 = v - hi.astype(np.float64)
    mid = r1.astype(np.float16)
    r2 = r1 - mid.astype(np.float64)
    lo = r2.astype(np.float16)
    return hi, mid, lo


def _coeffs(mean, scale, theta):
    """Per-gaussian coefficients g1..g6 (fp64), feature-scaled."""
    m = mean.astype(np.float64)
    s = scale.astype(np.float64)
    th = (1.0 + np.sin(theta.astype(np.float64)[:, 0])) * np.pi
    c, sn = np.cos(th), np.sin(th)
    is1 = 1.0 / s[:, 0] ** 2
    is2 = 1.0 / s[:, 1] ** 2
    A = c * c * is1 + sn * sn * is2
    B = c * sn * (is1 - is2)
    C = sn * sn * is1 + c * c * is2
    mx = m[:, 0] - 0.5
    my = m[:, 1] - 0.5
    # features are [4x'^2, 4x'y', 4y'^2, 2x', 2y', 1]
    g = np.stack([
        -A / 8.0,
        -B / 4.0,
        -C / 8.0,
        (A * mx + B * my) / 2.0,
        (B * mx + C * my) / 2.0,
        -0.5 * (A * mx * mx + 2.0 * B * mx * my + C * my * my),
    ], axis=0)  # (6, N)
    return g


def _features(pixels_flat):
    """Feature rows (6, P) fp64 from pixel coords (P, 2)."""
    p = pixels_flat.astype(np.float64)
    x = p[:, 0] - 0.5
    y = p[:, 1] - 0.5
    return np.stack([4*x*x, 4*x*y, 4*y*y, 2*x, 2*y, np.ones_like(x)], axis=0)


# Paired rows: (feature_index, f_piece, g_piece); pieces: 0=hi 1=mid 2=lo.
# 5 "big" features x 6 pairings + const x 2 = 32 rows. hh rows first
# (largest magnitudes accumulate/cancel early in the fp32 PSUM chain).
def _row_plan():
    plan = []
    big = [2, 1, 4, 0, 3]  # y^2, xy, y, x^2, x  (largest |g*f| first)
    for f in big:
        plan.append((f, 0, 0))   # hh
    plan.append((5, 0, 0))       # const * g_hi
    plan.append((5, 0, 1))       # const * g_mid
    for f in big:
        plan.append((f, 0, 1))   # hm
        plan.append((f, 1, 0))   # mh
    for f in big:
        plan.append((f, 1, 1))   # mm
        plan.append((f, 0, 2))   # hl
        plan.append((f, 2, 0))   # lh
    assert len(plan) == 32
    return plan


def _host_prep(mean, rgb, alpha, scale, theta, pixels):
    """Build per-core device operands."""
    plan = _row_plan()
    g = _coeffs(mean, scale, theta)              # (6, 128) fp64
    g_pieces = [_f16_split3(g[f]) for f in range(6)]   # list of (hi,mid,lo)

    # coef rows (32, 128) fp16
    coef = np.stack([g_pieces[f][gp] for (f, _fp, gp) in plan],
                    axis=0).astype(np.float16)

    rgba = (rgb[-1].astype(np.float64) * alpha[-1, 0].astype(np.float64))
    rgba_b = np.zeros((128, 4), dtype=np.float32)
    rgba_b[:, :3] = rgba.astype(np.float32)[None, :]

    # Pixel-block layout: within a core's 32768 pixels (p = q*256 + j),
    # block j holds pixels {q*256+j : q}.  F_sb[k, j*128+q] = F32[k, q*256+j].
    pix = np.asarray(pixels).reshape(H * W, 2)
    feats = []
    for core in range(N_CORES):
        pf = pix[core * PX_PER_CORE:(core + 1) * PX_PER_CORE]
        F = _features(pf)                        # (6, 32768) fp64
        f_pieces = [_f16_split3(F[f]) for f in range(6)]
        F32 = np.stack([f_pieces[f][fp] for (f, fp, _gp) in plan], axis=0)
        Fb = F32.reshape(32, 128, 256)           # [k, q, j]
        Fb = Fb.transpose(0, 2, 1)               # [k, j, q]
        Fsb = Fb.reshape(32, 256 * 128)          # partition k, col j*128+q
        feats.append(np.ascontiguousarray(Fsb.astype(np.float16)))
    return feats, coef, rgba_b


# ---------------------------------------------------------------------------
# Device kernel
# ---------------------------------------------------------------------------

_CACHE = {}


def _build_bass():
    import concourse.bacc as bacc
    import concourse.mybir as mybir
    from concourse.tile import TileContext

    fp16 = mybir.dt.float16
    f32 = mybir.dt.float32

    nc = bacc.Bacc("TRN2", target_bir_lowering=False)
    # chunk 0 split in half so round 0 starts sooner
    feat_d = [
        nc.dram_tensor("feat0a", [32, 1024], fp16, kind="ExternalInput"),
        nc.dram_tensor("feat0b", [32, 1024], fp16, kind="ExternalInput"),
    ] + [
        nc.dram_tensor(f"feat{t}", [32, 2048], fp16, kind="ExternalInput")
        for t in range(1, 16)
    ]
    coef_d = nc.dram_tensor("coef", [32, 128], fp16, kind="ExternalInput")
    rgba_d = nc.dram_tensor("rgba", [128, 4], f32, kind="ExternalInput")
    out_d = nc.dram_tensor("out", [128, 768], f32, kind="ExternalOutput")

    with TileContext(nc) as tc:
        with (
            tc.tile_pool(name="const", bufs=1) as cpool,
            tc.tile_pool(name="feat", bufs=1) as fpool,
            tc.tile_pool(name="psum", bufs=2, space="PSUM") as ppool,
            tc.tile_pool(name="splat", bufs=2) as spool,
            tc.tile_pool(name="scratch", bufs=2) as scpool,
            tc.tile_pool(name="acc", bufs=1) as apool,
        ):
            # warm the exp table while DMAs stream
            dummy = cpool.tile([128, 1], fp16, tag="dummy")
            nc.gpsimd.memset(dummy[:], 0)
            nc.scalar.activation(dummy[:], dummy[:],
                                 mybir.ActivationFunctionType.Exp)

            # constants go on the ACT hwdge ring, feat chunks on sync's,
            # with the first (smallest) chunks issued first
            g_sb = cpool.tile([32, 128], fp16, tag="gsb")
            nc.scalar.dma_start(g_sb[:], coef_d[:])
            rgba_sb = cpool.tile([128, 4], f32, tag="rgba")
            nc.scalar.dma_start(rgba_sb[:], rgba_d[:])

            ftiles = []
            for t, fd in enumerate(feat_d):
                ft = fpool.tile(list(fd.shape), fp16, tag=f"ft{t}")
                nc.sync.dma_start(ft[:], fd[:])
                ftiles.append(ft)

            S_big = apool.tile([128, 256], f32, tag="sbig")
            out_big = apool.tile([128, 768], f32, tag="outbig")

            for r in range(N_ROUNDS):
                ps = ppool.tile([128, 2048], f32, tag="ps")
                for i in range(16):
                    if r == 0:
                        lhsT = ftiles[i // 8][:, (i % 8) * 128:(i % 8 + 1) * 128]
                    else:
                        lhsT = ftiles[r + 1][:, i * 128:(i + 1) * 128]
                    nc.tensor.matmul(
                        ps[:, i * 128:(i + 1) * 128], lhsT, g_sb[:],
                    )
                sp = spool.tile([128, 2048], fp16, tag="sp")
                nc.scalar.activation(sp[:], ps[:],
                                     mybir.ActivationFunctionType.Exp)
                sp3 = sp[:].rearrange("p (i g) -> p i g", g=128)
                sc = scpool.tile([128, 1024], fp16, tag="sc")
                sc3 = sc[:].rearrange("p (i g) -> p i g", g=64)
                eng = nc.vector if r % 2 == 0 else nc.gpsimd
                eng.tensor_tensor(
                    sc3, sp3[:, :, 0:64], sp3[:, :, 64:128],
                    op=mybir.AluOpType.add,
                )
                nc.vector.tensor_reduce(
                    S_big[:, 16 * r:16 * (r + 1)], sc3,
                    axis=mybir.AxisListType.X, op=mybir.AluOpType.add,
                )

            ob3 = out_big[:].rearrange("p (j c) -> p j c", c=3)
            for c in range(3):
                nc.scalar.activation(
                    ob3[:, :, c], S_big[:],
                    mybir.ActivationFunctionType.Copy,
                    scale=rgba_sb[:, c:c + 1],
                )
            nc.sync.dma_start(out_d[:], out_big[:])

    nc.finalize()
    return nc


def _run(inputs, trace=False):
    from concourse.bass_utils import run_bass_kernel_spmd

    feats, coef, rgba_b = _host_prep(**inputs)
    if "nc" not in _CACHE:
        _CACHE["nc"] = _build_bass()
    nc = _CACHE["nc"]

    in_maps = []
    for core in range(N_CORES):
        fc = feats[core]
        m = {"feat0a": np.ascontiguousarray(fc[:, 0:1024]),
             "feat0b": np.ascontiguousarray(fc[:, 1024:2048])}
        for t in range(1, 16):
            m[f"feat{t}"] = np.ascontiguousarray(
                fc[:, t * 2048:(t + 1) * 2048])
        m["coef"] = coef
        m["rgba"] = rgba_b
        in_maps.append(m)

    res = run_bass_kernel_spmd(
        nc, in_maps, core_ids=list(range(N_CORES)), trace=trace,
    )
    shards = []
    for core in range(N_CORES):
        o = res.results[core]["out"]           # (128, 768) f32
        o = o.reshape(128, 256, 3)             # [q, j, c]
        o = o.reshape(64, 2, 256, 3)           # [row, half, j, c]
        shards.append(o.reshape(64, 512, 3))
    full = np.concatenate(shards, axis=0).astype(np.float32)
    return full, res


def kernel(mean, rgb, alpha, scale, theta, pixels):
    out, _ = _run(dict(mean=mean, rgb=rgb, alpha=alpha, scale=scale,
                       theta=theta, pixels=pixels))
    return out
